# Initial kernel scaffold
#
"""Causal self-attention (B=4, T=2048, C=1024, H=16, D=64) on 8 TRN2 NeuronCores.

Sharding: core c = (batch b = c//2, head-group hg = c%2 of 8 heads).
Launch-overhead-optimized layout: everything on the wire is bf16 and
deduplicated with on-device collectives:
  - x^T is pre-transposed on host; each core stages HALF of x[b]^T
    ([1024 C, 1024 T] bf16) and pair-AllGathers the other half.
  - W_attn / W_proj head-group sections are staged as quarters and
    quad-AllGathered across the 4 cores sharing each section.
  - Each core computes its partial y @ W_proj in fp32, the pair
    ReduceScatters (add) over tokens, and outputs [1024, 1024] bf16.
Host gather: out[b] = concat(core 2b tokens 0:1024, core 2b+1 tokens
1024:2048), upcast to fp32.

On-chip: QKV projections emit q^T/k^T bf16 [d, T] directly; v natural
[t, c'] bf16 with a ones column so PV accumulates softmax denominators;
flash causal attention in column-softmax layout (S^T blocks of
[128 keys, 512 queries], exp on ACT with 1/8 scale, diagonal-block
masking via precomputed bf16 0/1 masks on DVE); PV accumulates fp32 in
PSUM; normalize via reciprocal + gpsimd partition broadcast.
"""

import math
import sys

import numpy as np

for _p in ("/opt/trn_rl_repo", "/root/.axon_site/_ro/trn_rl_repo"):
    if _p not in sys.path:
        sys.path.append(_p)

import concourse.bass as bass
import concourse.tile as tile
from concourse import bacc, mybir
from concourse.bass_utils import run_bass_kernel_spmd

dt = mybir.dt
AF = mybir.ActivationFunctionType

B, T, C = 4, 2048, 1024
H, D = 16, 64
N_CORES = 8
S = 512           # per-core section width (8 heads * 64)
TH = 1024         # token half staged per core
TQ = 512          # query chunk
NTQ = T // TQ     # 4 query chunks
NCC = C // 128    # 8 contraction chunks
NTT = T // 128    # 16 token tiles
HC = H // 2       # 8 local heads

PAIRS = [[0, 1], [2, 3], [4, 5], [6, 7]]
QUADS = [[0, 2, 4, 6], [1, 3, 5, 7]]


def build_nc(reps: int = 1):
    nc = bacc.Bacc("TRN2", target_bir_lowering=False, debug=False,
                   num_devices=N_CORES)

    xt_d = nc.dram_tensor("xt", [C, TH], dt.bfloat16, kind="ExternalInput").ap()
    wq4_d = nc.dram_tensor("wq4", [C // 4, 3 * S], dt.bfloat16,
                           kind="ExternalInput").ap()
    wp4_d = nc.dram_tensor("wp4", [S // 4, C], dt.bfloat16,
                           kind="ExternalInput").ap()
    bqk_d = nc.dram_tensor("bqk", [128, 8], dt.float32, kind="ExternalInput").ap()
    vb_d = nc.dram_tensor("vbrow", [1, S], dt.bfloat16, kind="ExternalInput").ap()
    pb_d = nc.dram_tensor("pbrow", [1, C], dt.bfloat16, kind="ExternalInput").ap()
    mask_d = nc.dram_tensor("mask", [128, 896], dt.bfloat16,
                            kind="ExternalInput").ap()

    out_d = nc.dram_tensor("out", [TH, C], dt.bfloat16, kind="ExternalOutput").ap()

    with tile.TileContext(nc) as tc:
        if reps == 1:
            _emit(nc, tc, xt_d, wq4_d, wp4_d, bqk_d, vb_d, pb_d, mask_d, out_d)
        else:
            with tc.For_i(0, reps, 1):
                _emit(nc, tc, xt_d, wq4_d, wp4_d, bqk_d, vb_d, pb_d, mask_d,
                      out_d)
    return nc


def _emit(nc, tc, xt_d, wq4_d, wp4_d, bqk_d, vb_d, pb_d, mask_d, out_d):
    from contextlib import ExitStack

    es = ExitStack()
    with es:
        dram = es.enter_context(tc.tile_pool(name="dram", bufs=1, space="DRAM"))
        const = es.enter_context(tc.tile_pool(name="const", bufs=1))
        wsb = es.enter_context(tc.tile_pool(name="wsb", bufs=1))
        xsb = es.enter_context(tc.tile_pool(name="xsb", bufs=1))
        qkv_sb = es.enter_context(tc.tile_pool(name="qkv", bufs=1))
        yT_pool = es.enter_context(tc.tile_pool(name="yT", bufs=1))

        # ---- bounces + collectives (dedup staged bytes on device) ----
        xt_b = dram.tile([C, TH], dt.bfloat16)
        xt_ag = dram.tile([2 * C, TH], dt.bfloat16)   # [slab0; slab1]
        wq_b = dram.tile([C // 4, 3 * S], dt.bfloat16)
        wq_ag = dram.tile([C, 3 * S], dt.bfloat16)
        wp_b = dram.tile([S // 4, C], dt.bfloat16)
        wp_ag = dram.tile([S, C], dt.bfloat16)
        part = dram.tile([T, C], dt.bfloat16)         # partial y (pre-RS)
        rs_out = dram.tile([TH, C], dt.bfloat16)

        nc.sync.dma_start(xt_b[:], xt_d[:])
        nc.scalar.dma_start(wq_b[:], wq4_d[:])
        nc.gpsimd.dma_start(wp_b[:], wp4_d[:])
        nc.gpsimd.collective_compute(
            "AllGather", mybir.AluOpType.bypass, replica_groups=PAIRS,
            ins=[xt_b.opt()], outs=[xt_ag.opt()])
        nc.gpsimd.collective_compute(
            "AllGather", mybir.AluOpType.bypass, replica_groups=QUADS,
            ins=[wq_b.opt()], outs=[wq_ag.opt()])
        nc.gpsimd.collective_compute(
            "AllGather", mybir.AluOpType.bypass, replica_groups=QUADS,
            ins=[wp_b.opt()], outs=[wp_ag.opt()])

        # ---- constants ----
        mask = const.tile([128, 896], dt.bfloat16, tag="mask")
        nc.gpsimd.dma_start(mask[:], mask_d[:])
        bqk = const.tile([128, 8], dt.float32, tag="bqk")
        nc.gpsimd.dma_start(bqk[:], bqk_d[:])
        vbrow = const.tile([1, S], dt.bfloat16, tag="vbrow")
        nc.gpsimd.dma_start(vbrow[:], vb_d[:])
        pbrow = const.tile([1, C], dt.bfloat16, tag="pbrow")
        nc.gpsimd.dma_start(pbrow[:], pb_d[:])
        ones1 = const.tile([1, 128], dt.bfloat16, tag="ones1")
        nc.vector.memset(ones1[:], 1.0)
        onesv = const.tile([128, 8], dt.bfloat16, tag="onesv")
        nc.vector.memset(onesv[:], 1.0)

        # ---- SBUF weight + activation tiles ----
        wq = []
        for cc in range(NCC):
            w = wsb.tile([128, 3 * S], dt.bfloat16, tag=f"w{cc}", name=f"w{cc}")
            nc.gpsimd.dma_start(w[:], wq_ag[cc * 128:(cc + 1) * 128, :])
            wq.append(w)
        wp = []
        for cp in range(4):
            w = wsb.tile([128, C], dt.bfloat16, tag=f"wp{cp}", name=f"wp{cp}")
            nc.gpsimd.dma_start(w[:], wp_ag[cp * 128:(cp + 1) * 128, :])
            wp.append(w)
        xT = []
        for cc in range(NCC):
            t = xsb.tile([128, T], dt.bfloat16, tag=f"xT{cc}", name=f"xT{cc}")
            nc.gpsimd.dma_start(t[:, 0:TH], xt_ag[cc * 128:(cc + 1) * 128, :])
            nc.gpsimd.dma_start(t[:, TH:T], xt_ag[C + cc * 128:C + (cc + 1) * 128, :])
            xT.append(t)

        qT = [qkv_sb.tile([128, T], dt.bfloat16, tag=f"qT{i}", name=f"qT{i}")
              for i in range(4)]
        kT = [qkv_sb.tile([128, T], dt.bfloat16, tag=f"kT{i}", name=f"kT{i}")
              for i in range(4)]
        vt = [qkv_sb.tile([128, HC * (D + 1)], dt.bfloat16, tag=f"v{i}",
                          name=f"v{i}") for i in range(NTT)]

        # ============ Stage B: qkv projections ============
        with ExitStack() as ab:
            ps_mm = ab.enter_context(tc.tile_pool(name="ps_mm", bufs=3,
                                                  space="PSUM"))
            for t4 in range(4):                      # 512-token chunks
                for j in range(8):                   # q/k W-column tiles
                    pmm = ps_mm.tile([128, TQ], dt.float32)
                    for cc in range(NCC):
                        nc.tensor.matmul(
                            pmm[:], wq[cc][:, j * 128:(j + 1) * 128],
                            xT[cc][:, t4 * TQ:(t4 + 1) * TQ],
                            start=(cc == 0), stop=(cc == NCC - 1))
                    dest = qT[j] if j < 4 else kT[j - 4]
                    nc.scalar.activation(
                        dest[:, t4 * TQ:(t4 + 1) * TQ], pmm[:],
                        AF.Identity, bias=bqk[:, j:j + 1], scale=1.0)

            for g in range(NTT):                     # v (natural layout)
                pmm = ps_mm.tile([128, S], dt.float32)
                for cc in range(NCC):
                    nc.tensor.matmul(
                        pmm[:], xT[cc][:, g * 128:(g + 1) * 128],
                        wq[cc][:, 2 * S:3 * S],
                        start=(cc == 0), stop=False)
                nc.tensor.matmul(pmm[:], ones1[:], vbrow[:],
                                 start=False, stop=True)
                v3 = vt[g][:].rearrange("p (h e) -> p h e", e=D + 1)
                nc.scalar.copy(
                    v3[:, :, 0:D], pmm[:].rearrange("p (h e) -> p h e", e=D))
                nc.vector.tensor_copy(v3[:, :, D], onesv[:])

        yT = [yT_pool.tile([128, T], dt.bfloat16, tag=f"yT{i}", name=f"yT{i}")
              for i in range(4)]

        # ============ Stage C: causal flash attention ============
        with ExitStack() as at:
            pt_sb = at.enter_context(tc.tile_pool(name="pt", bufs=4))
            rc_sb = at.enter_context(tc.tile_pool(name="rc", bufs=2))
            ps_s = at.enter_context(tc.tile_pool(name="ps_s", bufs=3,
                                                 space="PSUM"))
            ps_y = at.enter_context(tc.tile_pool(name="ps_y", bufs=2,
                                                 space="PSUM"))

            work = []
            for h in range(HC):
                for qc in range(NTQ):
                    npair = (4 * qc + 4) // 2
                    for p in range(npair):
                        work.append((h, qc, p, npair))

            pt_of = {}
            y_of = {}

            def emit_s(idx):
                h, qc, p, npair = work[idx]
                ht, hp = h // 2, (h % 2) * D
                qs = qT[ht][hp:hp + D, qc * TQ:(qc + 1) * TQ]
                s_ps = ps_s.tile([128, 2 * TQ], dt.float32, tag="s",
                                 name=f"s{idx}")
                for half in range(2):
                    kb = 2 * p + half
                    nc.tensor.matmul(
                        s_ps[:, half * TQ:(half + 1) * TQ],
                        kT[ht][hp:hp + D, kb * 128:(kb + 1) * 128],
                        qs, start=True, stop=True)
                pt = pt_sb.tile([128, 2 * TQ], dt.bfloat16, tag="pt",
                                name=f"pt{idx}")
                nc.scalar.activation(pt[:], s_ps[:], AF.Exp,
                                     scale=1.0 / math.sqrt(D))
                for half in range(2):
                    kb = 2 * p + half
                    m = kb - 4 * qc
                    if m >= 0:
                        mo = 384 - 128 * m
                        nc.vector.tensor_mul(
                            pt[:, half * TQ:(half + 1) * TQ],
                            pt[:, half * TQ:(half + 1) * TQ],
                            mask[:, mo:mo + TQ])
                pt_of[idx] = pt

            def emit_pv(idx):
                h, qc, p, npair = work[idx]
                ht, hp = h // 2, (h % 2) * D
                if p == 0:
                    y_of[(h, qc)] = ps_y.tile([D + 1, TQ], dt.float32,
                                              tag="y", name=f"y{h}_{qc}")
                y_ps = y_of[(h, qc)]
                pt = pt_of.pop(idx)
                for half in range(2):
                    kb = 2 * p + half
                    nc.tensor.matmul(
                        y_ps[:],
                        vt[kb][:, h * (D + 1):(h + 1) * (D + 1)],
                        pt[:, half * TQ:(half + 1) * TQ],
                        start=(kb == 0), stop=(kb == 2 * npair - 1))
                if p == npair - 1:
                    rec = rc_sb.tile([1, TQ], dt.float32, tag="rec",
                                     name=f"rec{idx}")
                    nc.vector.reciprocal(rec[:], y_ps[D:D + 1, :])
                    bcast = rc_sb.tile([D, TQ], dt.float32, tag="bcast",
                                       name=f"bcast{idx}")
                    nc.gpsimd.partition_broadcast(bcast[:], rec[:])
                    nc.vector.tensor_mul(
                        yT[ht][hp:hp + D, qc * TQ:(qc + 1) * TQ],
                        y_ps[0:D, :], bcast[:])

            LOOKAHEAD = 2
            for j in range(min(LOOKAHEAD, len(work))):
                emit_s(j)
            for i in range(len(work)):
                if i + LOOKAHEAD < len(work):
                    emit_s(i + LOOKAHEAD)
                emit_pv(i)

        # ============ Stage D: output projection -> partial (fp32) ============
        with ExitStack() as od:
            o_sb = od.enter_context(tc.tile_pool(name="osb", bufs=3))
            ps_o = od.enter_context(tc.tile_pool(name="ps_o", bufs=2,
                                                 space="PSUM"))
            for g in range(NTT):
                p_o = ps_o.tile([128, C], dt.float32)
                for oc in range(2):
                    sl = slice(oc * 512, (oc + 1) * 512)
                    for cp in range(4):
                        nc.tensor.matmul(
                            p_o[:, sl], yT[cp][:, g * 128:(g + 1) * 128],
                            wp[cp][:, sl],
                            start=(cp == 0), stop=False)
                    nc.tensor.matmul(p_o[:, sl], ones1[:], pbrow[:, sl],
                                     start=False, stop=True)
                osb = o_sb.tile([128, C], dt.bfloat16)
                nc.scalar.copy(osb[:], p_o[:])
                nc.gpsimd.dma_start(part[g * 128:(g + 1) * 128, :], osb[:])

        # ============ Stage E: pair reduce-scatter + bf16 output ============
        with ExitStack() as oe:
            ob_sb = oe.enter_context(tc.tile_pool(name="ob", bufs=3))
            nc.gpsimd.collective_compute(
                "ReduceScatter", mybir.AluOpType.add, replica_groups=PAIRS,
                ins=[part.opt()], outs=[rs_out.opt()])
            nc.gpsimd.dma_start(out_d[:], rs_out[:])


def make_in_maps(x, W_attn, b_attn, W_proj, b_proj):
    import ml_dtypes

    bf16 = ml_dtypes.bfloat16
    x = np.asarray(x, dtype=np.float32)
    W_attn = np.asarray(W_attn, dtype=np.float32)
    b_attn = np.asarray(b_attn, dtype=np.float32)
    W_proj = np.asarray(W_proj, dtype=np.float32)
    b_proj = np.asarray(b_proj, dtype=np.float32)

    # combined diagonal-block mask: [:, 384-128m : 896-128m] gives the
    # pattern "valid iff j >= i + 128*m" for m in 0..3
    i = np.arange(128)[:, None]
    u = np.arange(896)[None, :]
    mask = (u >= i + 384).astype(bf16)

    in_maps = []
    for c in range(N_CORES):
        b, hg = divmod(c, 2)
        lo, hi = hg * S, (hg + 1) * S
        wqkv = np.concatenate(
            [W_attn[:, lo:hi], W_attn[:, C + lo:C + hi],
             W_attn[:, 2 * C + lo:2 * C + hi]], axis=1)
        q4 = c // 2
        bqk = np.stack(
            [b_attn[lo + t * 128:lo + (t + 1) * 128] for t in range(4)]
            + [b_attn[C + lo + t * 128:C + lo + (t + 1) * 128] for t in range(4)],
            axis=1)
        xt_half = x[b].T[:, hg * TH:(hg + 1) * TH]
        pbrow = (b_proj if hg == 0 else np.zeros_like(b_proj))[None, :]
        in_maps.append({
            "xt": np.ascontiguousarray(xt_half).astype(bf16),
            "wq4": np.ascontiguousarray(
                wqkv[q4 * 256:(q4 + 1) * 256, :]).astype(bf16),
            "wp4": np.ascontiguousarray(
                W_proj[lo + q4 * 128:lo + (q4 + 1) * 128, :]).astype(bf16),
            "bqk": np.ascontiguousarray(bqk),
            "vbrow": b_attn[2 * C + lo:2 * C + hi][None, :].astype(bf16),
            "pbrow": pbrow.astype(bf16),
            "mask": mask,
        })
    return in_maps


_NC_CACHE = {}


def _get_nc(reps: int = 1):
    if reps not in _NC_CACHE:
        nc = build_nc(reps)
        nc.finalize()
        _NC_CACHE[reps] = nc
    return _NC_CACHE[reps]


def kernel(x, W_attn, b_attn, W_proj, b_proj):
    in_maps = make_in_maps(x, W_attn, b_attn, W_proj, b_proj)
    nc = _get_nc(1)
    res = run_bass_kernel_spmd(nc, in_maps, list(range(N_CORES)))
    out = np.empty((B, T, C), dtype=np.float32)
    for b in range(B):
        out[b, 0:TH] = res.results[2 * b]["out"].astype(np.float32)
        out[b, TH:T] = res.results[2 * b + 1]["out"].astype(np.float32)
    return out



# revision 1
# speedup vs baseline: 2.9168x; 2.9168x over previous
"""Causal self-attention (B=4, T=2048, C=1024, H=16, D=64) on 8 TRN2 NeuronCores.

Sharding: core c = (batch b = c//2, head-group hg = c%2 of 8 heads).
Launch-overhead-optimized layout: everything on the wire is bf16 and
deduplicated with on-device collectives:
  - x^T is pre-transposed on host; each core stages HALF of x[b]^T
    ([1024 C, 1024 T] bf16) and pair-AllGathers the other half.
  - W_attn / W_proj head-group sections are staged as quarters and
    quad-AllGathered across the 4 cores sharing each section.
  - Each core computes its partial y @ W_proj in fp32, the pair
    ReduceScatters (add) over tokens, and outputs [1024, 1024] bf16.
Host gather: out[b] = concat(core 2b tokens 0:1024, core 2b+1 tokens
1024:2048), upcast to fp32.

On-chip: QKV projections emit q^T/k^T bf16 [d, T] directly; v natural
[t, c'] bf16 with a ones column so PV accumulates softmax denominators;
flash causal attention in column-softmax layout (S^T blocks of
[128 keys, 512 queries], exp on ACT with 1/8 scale, diagonal-block
masking via precomputed bf16 0/1 masks on DVE); PV accumulates fp32 in
PSUM; normalize via reciprocal + gpsimd partition broadcast.
"""

import math
import sys

import numpy as np

for _p in ("/opt/trn_rl_repo", "/root/.axon_site/_ro/trn_rl_repo"):
    if _p not in sys.path:
        sys.path.append(_p)

import concourse.bass as bass
import concourse.tile as tile
from concourse import bacc, mybir
from concourse.bass_utils import run_bass_kernel_spmd

dt = mybir.dt
AF = mybir.ActivationFunctionType

B, T, C = 4, 2048, 1024
H, D = 16, 64
N_CORES = 8
S = 512           # per-core section width (8 heads * 64)
TH = 1024         # token half staged per core
TQ = 512          # query chunk
NTQ = T // TQ     # 4 query chunks
NCC = C // 128    # 8 contraction chunks
NTT = T // 128    # 16 token tiles
HC = H // 2       # 8 local heads

PAIRS = [[0, 1], [2, 3], [4, 5], [6, 7]]
QUADS = [[0, 2, 4, 6], [1, 3, 5, 7]]


def build_nc(reps: int = 1):
    nc = bacc.Bacc("TRN2", target_bir_lowering=False, debug=False,
                   num_devices=N_CORES)

    xt_d = nc.dram_tensor("xt", [C, TH], dt.bfloat16, kind="ExternalInput").ap()
    wq4_d = nc.dram_tensor("wq4", [C // 4, 3 * S], dt.bfloat16,
                           kind="ExternalInput").ap()
    wp4_d = nc.dram_tensor("wp4", [S // 4, C], dt.bfloat16,
                           kind="ExternalInput").ap()
    bqk_d = nc.dram_tensor("bqk", [128, 8], dt.float32, kind="ExternalInput").ap()
    vb_d = nc.dram_tensor("vbrow", [1, S], dt.bfloat16, kind="ExternalInput").ap()
    pb_d = nc.dram_tensor("pbrow", [1, C], dt.bfloat16, kind="ExternalInput").ap()
    mask_d = nc.dram_tensor("mask", [128, 896], dt.bfloat16,
                            kind="ExternalInput").ap()

    out_d = nc.dram_tensor("out", [TH, C], dt.bfloat16, kind="ExternalOutput").ap()

    with tile.TileContext(nc) as tc:
        if reps == 1:
            _emit(nc, tc, xt_d, wq4_d, wp4_d, bqk_d, vb_d, pb_d, mask_d, out_d)
        else:
            with tc.For_i(0, reps, 1):
                _emit(nc, tc, xt_d, wq4_d, wp4_d, bqk_d, vb_d, pb_d, mask_d,
                      out_d)
    return nc


def _emit(nc, tc, xt_d, wq4_d, wp4_d, bqk_d, vb_d, pb_d, mask_d, out_d):
    from contextlib import ExitStack

    es = ExitStack()
    with es:
        dram = es.enter_context(tc.tile_pool(name="dram", bufs=1, space="DRAM"))
        const = es.enter_context(tc.tile_pool(name="const", bufs=1))
        wsb = es.enter_context(tc.tile_pool(name="wsb", bufs=1))
        xsb = es.enter_context(tc.tile_pool(name="xsb", bufs=1))
        qkv_sb = es.enter_context(tc.tile_pool(name="qkv", bufs=1))
        yT_pool = es.enter_context(tc.tile_pool(name="yT", bufs=1))

        # ---- bounces + collectives (dedup staged bytes on device) ----
        xt_b = dram.tile([C, TH], dt.bfloat16)
        xt_ag = dram.tile([2 * C, TH], dt.bfloat16)   # [slab0; slab1]
        wq_b = dram.tile([C // 4, 3 * S], dt.bfloat16)
        wq_ag = dram.tile([C, 3 * S], dt.bfloat16)
        wp_b = dram.tile([S // 4, C], dt.bfloat16)
        wp_ag = dram.tile([S, C], dt.bfloat16)
        part = dram.tile([T, C], dt.bfloat16)         # partial y (pre-RS)
        rs_out = dram.tile([TH, C], dt.bfloat16)

        nc.sync.dma_start(xt_b[:], xt_d[:])
        nc.scalar.dma_start(wq_b[:], wq4_d[:])
        nc.gpsimd.dma_start(wp_b[:], wp4_d[:])
        nc.gpsimd.collective_compute(
            "AllGather", mybir.AluOpType.bypass, replica_groups=PAIRS,
            ins=[xt_b.opt()], outs=[xt_ag.opt()])
        nc.gpsimd.collective_compute(
            "AllGather", mybir.AluOpType.bypass, replica_groups=QUADS,
            ins=[wq_b.opt()], outs=[wq_ag.opt()])
        nc.gpsimd.collective_compute(
            "AllGather", mybir.AluOpType.bypass, replica_groups=QUADS,
            ins=[wp_b.opt()], outs=[wp_ag.opt()])

        # ---- constants ----
        mask = const.tile([128, 896], dt.bfloat16, tag="mask")
        nc.gpsimd.dma_start(mask[:], mask_d[:])
        bqk = const.tile([128, 8], dt.float32, tag="bqk")
        nc.gpsimd.dma_start(bqk[:], bqk_d[:])
        vbrow = const.tile([1, S], dt.bfloat16, tag="vbrow")
        nc.gpsimd.dma_start(vbrow[:], vb_d[:])
        pbrow = const.tile([1, C], dt.bfloat16, tag="pbrow")
        nc.gpsimd.dma_start(pbrow[:], pb_d[:])
        ones1 = const.tile([1, 128], dt.bfloat16, tag="ones1")
        nc.vector.memset(ones1[:], 1.0)
        onesv = const.tile([128, 8], dt.bfloat16, tag="onesv")
        nc.vector.memset(onesv[:], 1.0)

        # ---- SBUF weight + activation tiles ----
        wq = []
        for cc in range(NCC):
            w = wsb.tile([128, 3 * S], dt.bfloat16, tag=f"w{cc}", name=f"w{cc}")
            nc.gpsimd.dma_start(w[:], wq_ag[cc * 128:(cc + 1) * 128, :])
            wq.append(w)
        wp = []
        for cp in range(4):
            w = wsb.tile([128, C], dt.bfloat16, tag=f"wp{cp}", name=f"wp{cp}")
            nc.gpsimd.dma_start(w[:], wp_ag[cp * 128:(cp + 1) * 128, :])
            wp.append(w)
        xT = []
        for cc in range(NCC):
            t = xsb.tile([128, T], dt.bfloat16, tag=f"xT{cc}", name=f"xT{cc}")
            nc.gpsimd.dma_start(t[:, 0:TH], xt_ag[cc * 128:(cc + 1) * 128, :])
            nc.gpsimd.dma_start(t[:, TH:T], xt_ag[C + cc * 128:C + (cc + 1) * 128, :])
            xT.append(t)

        qT = [qkv_sb.tile([128, T], dt.bfloat16, tag=f"qT{i}", name=f"qT{i}")
              for i in range(4)]
        kT = [qkv_sb.tile([128, T], dt.bfloat16, tag=f"kT{i}", name=f"kT{i}")
              for i in range(4)]
        vt = [qkv_sb.tile([128, HC * (D + 1)], dt.bfloat16, tag=f"v{i}",
                          name=f"v{i}") for i in range(NTT)]

        # ============ Stage B: qkv projections ============
        with ExitStack() as ab:
            ps_mm = ab.enter_context(tc.tile_pool(name="ps_mm", bufs=3,
                                                  space="PSUM"))
            for t4 in range(4):                      # 512-token chunks
                for j in range(8):                   # q/k W-column tiles
                    pmm = ps_mm.tile([128, TQ], dt.float32)
                    for cc in range(NCC):
                        nc.tensor.matmul(
                            pmm[:], wq[cc][:, j * 128:(j + 1) * 128],
                            xT[cc][:, t4 * TQ:(t4 + 1) * TQ],
                            start=(cc == 0), stop=(cc == NCC - 1))
                    dest = qT[j] if j < 4 else kT[j - 4]
                    nc.scalar.activation(
                        dest[:, t4 * TQ:(t4 + 1) * TQ], pmm[:],
                        AF.Identity, bias=bqk[:, j:j + 1], scale=1.0)

            for g in range(NTT):                     # v (natural layout)
                pmm = ps_mm.tile([128, S], dt.float32)
                for cc in range(NCC):
                    nc.tensor.matmul(
                        pmm[:], xT[cc][:, g * 128:(g + 1) * 128],
                        wq[cc][:, 2 * S:3 * S],
                        start=(cc == 0), stop=False)
                nc.tensor.matmul(pmm[:], ones1[:], vbrow[:],
                                 start=False, stop=True)
                v3 = vt[g][:].rearrange("p (h e) -> p h e", e=D + 1)
                nc.scalar.copy(
                    v3[:, :, 0:D], pmm[:].rearrange("p (h e) -> p h e", e=D))
                nc.vector.tensor_copy(v3[:, :, D], onesv[:])

        yT = [yT_pool.tile([128, T], dt.bfloat16, tag=f"yT{i}", name=f"yT{i}")
              for i in range(4)]

        # ============ Stage C: causal flash attention ============
        with ExitStack() as at:
            pt_sb = at.enter_context(tc.tile_pool(name="pt", bufs=4))
            rc_sb = at.enter_context(tc.tile_pool(name="rc", bufs=2))
            ps_s = at.enter_context(tc.tile_pool(name="ps_s", bufs=3,
                                                 space="PSUM"))
            ps_y = at.enter_context(tc.tile_pool(name="ps_y", bufs=2,
                                                 space="PSUM"))

            work = []
            for h in range(HC):
                for qc in range(NTQ):
                    npair = (4 * qc + 4) // 2
                    for p in range(npair):
                        work.append((h, qc, p, npair))

            pt_of = {}
            y_of = {}

            def emit_s(idx):
                h, qc, p, npair = work[idx]
                ht, hp = h // 2, (h % 2) * D
                qs = qT[ht][hp:hp + D, qc * TQ:(qc + 1) * TQ]
                s_ps = ps_s.tile([128, 2 * TQ], dt.float32, tag="s",
                                 name=f"s{idx}")
                for half in range(2):
                    kb = 2 * p + half
                    nc.tensor.matmul(
                        s_ps[:, half * TQ:(half + 1) * TQ],
                        kT[ht][hp:hp + D, kb * 128:(kb + 1) * 128],
                        qs, start=True, stop=True)
                pt = pt_sb.tile([128, 2 * TQ], dt.bfloat16, tag="pt",
                                name=f"pt{idx}")
                nc.scalar.activation(pt[:], s_ps[:], AF.Exp,
                                     scale=1.0 / math.sqrt(D))
                for half in range(2):
                    kb = 2 * p + half
                    m = kb - 4 * qc
                    if m >= 0:
                        mo = 384 - 128 * m
                        nc.vector.tensor_mul(
                            pt[:, half * TQ:(half + 1) * TQ],
                            pt[:, half * TQ:(half + 1) * TQ],
                            mask[:, mo:mo + TQ])
                pt_of[idx] = pt

            def emit_pv(idx):
                h, qc, p, npair = work[idx]
                ht, hp = h // 2, (h % 2) * D
                if p == 0:
                    y_of[(h, qc)] = ps_y.tile([D + 1, TQ], dt.float32,
                                              tag="y", name=f"y{h}_{qc}")
                y_ps = y_of[(h, qc)]
                pt = pt_of.pop(idx)
                for half in range(2):
                    kb = 2 * p + half
                    nc.tensor.matmul(
                        y_ps[:],
                        vt[kb][:, h * (D + 1):(h + 1) * (D + 1)],
                        pt[:, half * TQ:(half + 1) * TQ],
                        start=(kb == 0), stop=(kb == 2 * npair - 1))
                if p == npair - 1:
                    rec = rc_sb.tile([1, TQ], dt.float32, tag="rec",
                                     name=f"rec{idx}")
                    nc.vector.reciprocal(rec[:], y_ps[D:D + 1, :])
                    bcast = rc_sb.tile([D, TQ], dt.float32, tag="bcast",
                                       name=f"bcast{idx}")
                    nc.gpsimd.partition_broadcast(bcast[:], rec[:])
                    nc.vector.tensor_mul(
                        yT[ht][hp:hp + D, qc * TQ:(qc + 1) * TQ],
                        y_ps[0:D, :], bcast[:])

            LOOKAHEAD = 2
            for j in range(min(LOOKAHEAD, len(work))):
                emit_s(j)
            for i in range(len(work)):
                if i + LOOKAHEAD < len(work):
                    emit_s(i + LOOKAHEAD)
                emit_pv(i)

        # ============ Stage D: output projection -> partial (fp32) ============
        with ExitStack() as od:
            o_sb = od.enter_context(tc.tile_pool(name="osb", bufs=3))
            ps_o = od.enter_context(tc.tile_pool(name="ps_o", bufs=2,
                                                 space="PSUM"))
            for g in range(NTT):
                p_o = ps_o.tile([128, C], dt.float32)
                for oc in range(2):
                    sl = slice(oc * 512, (oc + 1) * 512)
                    for cp in range(4):
                        nc.tensor.matmul(
                            p_o[:, sl], yT[cp][:, g * 128:(g + 1) * 128],
                            wp[cp][:, sl],
                            start=(cp == 0), stop=False)
                    nc.tensor.matmul(p_o[:, sl], ones1[:], pbrow[:, sl],
                                     start=False, stop=True)
                osb = o_sb.tile([128, C], dt.bfloat16)
                nc.scalar.copy(osb[:], p_o[:])
                nc.gpsimd.dma_start(part[g * 128:(g + 1) * 128, :], osb[:])

        # ============ Stage E: pair reduce-scatter + bf16 output ============
        with ExitStack() as oe:
            ob_sb = oe.enter_context(tc.tile_pool(name="ob", bufs=3))
            nc.gpsimd.collective_compute(
                "ReduceScatter", mybir.AluOpType.add, replica_groups=PAIRS,
                ins=[part.opt()], outs=[rs_out.opt()])
            nc.gpsimd.dma_start(out_d[:], rs_out[:])


def make_in_maps(x, W_attn, b_attn, W_proj, b_proj):
    import ml_dtypes

    bf16 = ml_dtypes.bfloat16
    x = np.asarray(x, dtype=np.float32)
    W_attn = np.asarray(W_attn, dtype=np.float32)
    b_attn = np.asarray(b_attn, dtype=np.float32)
    W_proj = np.asarray(W_proj, dtype=np.float32)
    b_proj = np.asarray(b_proj, dtype=np.float32)

    # combined diagonal-block mask: [:, 384-128m : 896-128m] gives the
    # pattern "valid iff j >= i + 128*m" for m in 0..3
    i = np.arange(128)[:, None]
    u = np.arange(896)[None, :]
    mask = (u >= i + 384).astype(bf16)

    in_maps = []
    for c in range(N_CORES):
        b, hg = divmod(c, 2)
        lo, hi = hg * S, (hg + 1) * S
        wqkv = np.concatenate(
            [W_attn[:, lo:hi], W_attn[:, C + lo:C + hi],
             W_attn[:, 2 * C + lo:2 * C + hi]], axis=1)
        q4 = c // 2
        bqk = np.stack(
            [b_attn[lo + t * 128:lo + (t + 1) * 128] for t in range(4)]
            + [b_attn[C + lo + t * 128:C + lo + (t + 1) * 128] for t in range(4)],
            axis=1)
        xt_half = x[b].T[:, hg * TH:(hg + 1) * TH]
        pbrow = (b_proj if hg == 0 else np.zeros_like(b_proj))[None, :]
        in_maps.append({
            "xt": np.ascontiguousarray(xt_half).astype(bf16),
            "wq4": np.ascontiguousarray(
                wqkv[q4 * 256:(q4 + 1) * 256, :]).astype(bf16),
            "wp4": np.ascontiguousarray(
                W_proj[lo + q4 * 128:lo + (q4 + 1) * 128, :]).astype(bf16),
            "bqk": np.ascontiguousarray(bqk),
            "vbrow": b_attn[2 * C + lo:2 * C + hi][None, :].astype(bf16),
            "pbrow": pbrow.astype(bf16),
            "mask": mask,
        })
    return in_maps


_NC_CACHE = {}


def _get_nc(reps: int = 1):
    if reps not in _NC_CACHE:
        nc = build_nc(reps)
        nc.finalize()
        _NC_CACHE[reps] = nc
    return _NC_CACHE[reps]


def kernel(x, W_attn, b_attn, W_proj, b_proj):
    in_maps = make_in_maps(x, W_attn, b_attn, W_proj, b_proj)
    nc = _get_nc(1)
    res = run_bass_kernel_spmd(nc, in_maps, list(range(N_CORES)))
    out = np.empty((B, T, C), dtype=np.float32)
    for b in range(B):
        out[b, 0:TH] = res.results[2 * b]["out"].astype(np.float32)
        out[b, TH:T] = res.results[2 * b + 1]["out"].astype(np.float32)
    return out

